# revision 33
# baseline (speedup 1.0000x reference)
"""Trainium2 Bass kernel for GQA attention (B=2, S=2048, D=2048, H=16, HK=4).

Sharding: 8 devices = batch(2) x kv-groups(4). Each device owns one batch
element and one GQA group (4 q-heads + 1 kv-head): wq/wk/wv column-parallel,
wo row-parallel (host sums the 4 partials per batch element).

Device kernel:
  - QKV projection in fp8e4 DoubleRow (K=256 per matmul): x and the QKV
    weights are quantized host-side; the q/k weight blocks are pre-scaled
    by 128 (so fp8 never sees denormals) with the 1/128 folded into the
    RoPE cos/sin tables and biases. v's block is unscaled (its error is
    attenuated by softmax averaging).
  - RoPE on DVE with an even/odd dk permutation folded into the weights
    host-side (partitions 0:64 = real, 64:128 = imag); outputs bf16.
  - scores computed transposed [sk, sq] in bf16 so softmax needs no
    transposes; exp on ACT reads PSUM directly and writes fp8 e-tiles in
    sk-tile PAIRS; no max-subtraction (scores are O(1) here).
  - A@V and the denominator one-hot matmuls run fp8 DoubleRow over the
    sk-tile pairs (2x the fp32r rate, measured).
  - denominators: per-head one-hot DR matmuls accumulate row-sums into a
    single PSUM bank; one 4-row reciprocal; per-head broadcast via a K=4
    one-hot matmul.
  - causal: upper-triangle tiles skipped; diagonal pairs get an additive
    -1e9 mask before exp (which also zeroes the pair's invalid columns).
  - wo row-parallel fp32r matmul on device; host adds wo_b and reduces.
"""

import math

import ml_dtypes
import numpy as np

import concourse.bacc as bacc
import concourse.tile as tile
from concourse import mybir
from concourse.bass_utils import run_bass_kernel_spmd

B, S, D = 2, 2048, 2048
H, HK, DK = 16, 4, 128
REP = H // HK  # 4 q-heads per kv head
NDEV = 8
P = 128
CH = 512            # s-chunk (matmul moving size)
ND = D // P         # 16 d-tiles
NKP = ND // 2       # 8 DoubleRow k-pairs
F32 = mybir.dt.float32
F32R = mybir.dt.float32r
BF16 = mybir.dt.bfloat16
F8 = mybir.dt.float8e4
DR = mybir.MatmulPerfMode.DoubleRow
NEG = -1.0e9
WSC = 128.0         # host-side scale on q/k weight blocks (fp8 range use)
ESH = -2.0          # exp shift: e' = exp(s - 2); cancels in normalization.
                    # With k mean-centered, scores stay in [-8.3, 7.0], so
                    # e' <= ~160 < fp8e4 max 240 and row-maxes stay normal.
TP = 256            # exact-prefix length: the first TP query rows average
                    # too few positions to attenuate fp8 noise, so their
                    # q/k/v come from a bf16 projection and their attention
                    # pair runs in bf16.


_ABL = {"pfx": True, "ctr": True, "avdr": True}  # ablation knobs (timing expts)


def _build(s_len=S, reps=1):
    """Build the per-device Bass program (SPMD: same program on all cores).

    reps>1 repeats the whole computation (timing only)."""
    abl_pfx, abl_ctr, abl_avdr = _ABL["pfx"], _ABL["ctr"], _ABL["avdr"]
    nch = s_len // CH          # s-chunks
    scale = 1.0 / math.sqrt(DK)

    nc = bacc.Bacc("TRN2", target_bir_lowering=False, debug=False,
                   enable_asserts=False, num_devices=1)
    xT8 = nc.dram_tensor("xT8", [P, nch * ND * CH], F8, kind="ExternalInput").ap()
    W8 = nc.dram_tensor("W8", [P, 6 * D], F8, kind="ExternalInput").ap()
    XB = nc.dram_tensor("XB", [P, ND * TP], BF16, kind="ExternalInput").ap()
    WB = nc.dram_tensor("WB", [P, 6 * D], BF16, kind="ExternalInput").ap()
    ONEHB = nc.dram_tensor("ONEHB", [P, REP * 16], BF16, kind="ExternalInput").ap()
    woT = nc.dram_tensor("woT", [REP * DK, D], BF16, kind="ExternalInput").ap()
    CSt = nc.dram_tensor("CS", [P, s_len], F32, kind="ExternalInput").ap()
    SCt = nc.dram_tensor("SC", [P, s_len], F32, kind="ExternalInput").ap()
    MBt = nc.dram_tensor("MB", [P, 896], F32, kind="ExternalInput").ap()
    IDt = nc.dram_tensor("ID", [P, P], F32, kind="ExternalInput").ap()
    BIAS = nc.dram_tensor("BIAS", [P, 6], F32, kind="ExternalInput").ap()
    BIAS2 = nc.dram_tensor("BIAS2", [P, 6], F32, kind="ExternalInput").ap()
    ONEH8 = nc.dram_tensor("ONEH8", [P, REP * 2 * 16], F8, kind="ExternalInput").ap()
    ONEH4 = nc.dram_tensor("ONEH4", [REP, REP * P], F32R, kind="ExternalInput").ap()
    out = nc.dram_tensor("out", [s_len, D], BF16, kind="ExternalOutput").ap()

    with tile.TileContext(nc) as tc:
      for _rep in range(reps):
        with tc.tile_pool(name="consts", bufs=1) as consts, \
             tc.tile_pool(name="qkv", bufs=1) as qkpool:
            cs_sb = consts.tile([P, s_len], F32)
            sc_sb = consts.tile([P, s_len], F32)
            mb_sb = consts.tile([P, 896], F32)
            id_sb = consts.tile([P, P], F32)
            bias_sb = consts.tile([P, 6], F32)
            bias2_sb = consts.tile([P, 6], F32)
            oneh8_sb = consts.tile([P, REP, 2, 16], F8)
            oneh4_sb = consts.tile([REP, REP * P], F32R)
            esh_sb = consts.tile([P, 1], F32)
            nc.vector.memset(esh_sb, ESH)
            onehb_sb = consts.tile([P, REP, 16], BF16)

            qk_sb = qkpool.tile([P, 5 * s_len], BF16)  # roped q 0..3, k at 4
            v_sb = qkpool.tile([P, ND, P], F8)         # [s-in-tile, sk-tile, dk]
            vb_sb = qkpool.tile([P, TP // P, P], BF16)  # bf16 copy of v tiles 0-1

            # ---------------- Phase P: QKV projection (fp8 DR) + RoPE + vT
            with tc.tile_pool(name="xh", bufs=2) as xpool, \
                 tc.tile_pool(name="wst", bufs=1) as wpool, \
                 tc.tile_pool(name="rope", bufs=3) as rpool, \
                 tc.tile_pool(name="vT", bufs=1) as vtpool, \
                 tc.tile_pool(name="pp", bufs=4, space="PSUM") as pps, \
                 tc.tile_pool(name="pt", bufs=2, space="PSUM") as pts:
                vT_sb = vtpool.tile([P, s_len], F32)
                w8_sb = wpool.tile([P, 6, NKP, 2, P], F8)
                wb_sb = wpool.tile([P, 6, ND, P], BF16)
                xb_sb = wpool.tile([P, ND, TP], BF16)

                def load_xq(c):
                    xq = xpool.tile([P, NKP, 2, CH], F8, tag="x", name=f"xq{c}")
                    nc.sync.dma_start(
                        out=xq, in_=xT8[:, c * ND * CH:(c + 1) * ND * CH])
                    return xq

                def load_tabs(c):
                    nc.scalar.dma_start(out=cs_sb[:, c * CH:(c + 1) * CH],
                                        in_=CSt[:, c * CH:(c + 1) * CH])
                    nc.scalar.dma_start(out=sc_sb[:, c * CH:(c + 1) * CH],
                                        in_=SCt[:, c * CH:(c + 1) * CH])

                def emit_rope(ps, m, col0, fn):
                    # RoPE: partitions 0:64 real (qr), 64:128 imag.
                    # U[0:64]=(qr+b0)cos  U[64:]=(qr+b0)sin
                    # V[0:64]=(qi+b1)sin  V[64:]=(qi+b1)cos
                    # cos/sin tables carry the 1/WSC de-scale.
                    cs_c = cs_sb[:, col0:col0 + fn]
                    sc_c = sc_sb[:, col0:col0 + fn]
                    add, mult = mybir.AluOpType.add, mybir.AluOpType.mult
                    u = rpool.tile([P, CH], F32, tag="p1")
                    v = rpool.tile([P, CH], F32, tag="p2")
                    nc.vector.scalar_tensor_tensor(
                        u[0:64, 0:fn], ps[0:64, 0:fn], bias_sb[0:64, m:m + 1],
                        cs_c[0:64], op0=add, op1=mult)
                    nc.vector.scalar_tensor_tensor(
                        u[64:128, 0:fn], ps[0:64, 0:fn],
                        bias2_sb[64:128, m:m + 1],
                        cs_c[64:128], op0=add, op1=mult)
                    nc.vector.scalar_tensor_tensor(
                        v[0:64, 0:fn], ps[64:128, 0:fn],
                        bias2_sb[0:64, m:m + 1],
                        sc_c[0:64], op0=add, op1=mult)
                    nc.vector.scalar_tensor_tensor(
                        v[64:128, 0:fn], ps[64:128, 0:fn],
                        bias_sb[64:128, m:m + 1],
                        sc_c[64:128], op0=add, op1=mult)
                    dst = qk_sb[:, m * s_len + col0: m * s_len + col0 + fn]
                    nc.vector.tensor_sub(dst[0:64], u[0:64, 0:fn], v[0:64, 0:fn])
                    nc.vector.tensor_add(dst[64:128], u[64:128, 0:fn],
                                         v[64:128, 0:fn])

                nkm = wpool.tile([P, 1], F32)

                def center_k(col0, fn):
                    # scores_kj -> (k_k - kbar).q_j: a per-column shift that
                    # cancels in softmax normalization but tames the exp
                    # range so e' fits fp8e4. kbar is chunk-0's k mean (any
                    # fixed vector gives an exact-cancelling shift).
                    ksl = qk_sb[:, 4 * s_len + col0: 4 * s_len + col0 + fn]
                    nc.vector.tensor_scalar(ksl, ksl, nkm, None,
                                            op0=mybir.AluOpType.add)

                def emit_prefix():
                    # exact prefix: bf16 QKV for positions < TP overwrites
                    # the fp8-derived q/k/v there (short attention rows
                    # can't average away fp8 noise).
                    for m in range(6):
                        ps = pps.tile([P, TP], F32, tag="pp", name=f"pfx{m}")
                        for dt in range(ND):
                            nc.tensor.matmul(
                                ps, wb_sb[:, m, dt, :], xb_sb[:, dt, :],
                                start=(dt == 0), stop=(dt == ND - 1))
                        if m < 5:
                            emit_rope(ps, m, 0, TP)
                            if m == 4 and abl_ctr:
                                center_k(0, TP)
                        else:
                            nc.scalar.add(out=vT_sb[:, 0:TP], in_=ps,
                                          add=bias_sb[:, m:m + 1])
                    for tt in range(TP // P):
                        ptr = pts.tile([P, P], F32, tag="pt")
                        nc.tensor.transpose(ptr, vT_sb[:, tt * P:(tt + 1) * P],
                                            id_sb)
                        nc.any.tensor_copy(v_sb[:, tt, :], ptr)
                        nc.any.tensor_copy(vb_sb[:, tt, :], ptr)

                # W8 is only 1.5MB; x chunks are 1MB each. Interleave the
                # first x chunk with W8 so matmuls can start ~3us in.
                xq = load_xq(0)
                nc.scalar.dma_start(out=w8_sb, in_=W8)
                nc.scalar.dma_start(out=bias_sb, in_=BIAS)
                nc.scalar.dma_start(out=bias2_sb, in_=BIAS2)
                load_tabs(0)
                nc.scalar.dma_start(out=id_sb, in_=IDt)
                if abl_pfx:
                    nc.scalar.dma_start(out=wb_sb, in_=WB)
                    nc.scalar.dma_start(out=xb_sb, in_=XB)

                for c in range(nch):
                    if c > 0:
                        xq = load_xq(c)
                        load_tabs(c)
                    if c == (1 if nch > 1 else 0):
                        nc.sync.dma_start(out=mb_sb, in_=MBt)
                        nc.sync.dma_start(out=oneh8_sb, in_=ONEH8)
                        nc.sync.dma_start(out=oneh4_sb, in_=ONEH4)
                        nc.sync.dma_start(out=onehb_sb, in_=ONEHB)
                    # chunk 0's first TP columns come from the exact prefix;
                    # skip them in the fp8 pass.
                    cf0 = TP if (c == 0 and abl_pfx) else 0
                    cw = CH - cf0
                    for m in range(6):
                        ps = pps.tile([P, CH], F32, tag="pp")
                        for kp in range(NKP):
                            nc.tensor.matmul(
                                ps[:, 0:cw], w8_sb[:, m, kp, :, :],
                                xq[:, kp, :, cf0:CH],
                                start=(kp == 0), stop=(kp == NKP - 1),
                                perf_mode=DR)
                        if m < 5:
                            emit_rope(ps, m, c * CH + cf0, cw)
                            if m == 4 and abl_ctr:
                                if c == 0:
                                    ksl = qk_sb[:, 4 * s_len + cf0:
                                                4 * s_len + CH]
                                    nc.vector.tensor_reduce(
                                        nkm, ksl, axis=mybir.AxisListType.X,
                                        op=mybir.AluOpType.add, negate=True)
                                    nc.vector.tensor_scalar(
                                        nkm, nkm, 1.0 / cw, None,
                                        op0=mybir.AluOpType.mult)
                                center_k(c * CH + cf0, cw)
                        else:
                            nc.scalar.add(out=vT_sb[:, c * CH + cf0:(c + 1) * CH],
                                          in_=ps[:, 0:cw],
                                          add=bias_sb[:, m:m + 1])
                    for tt in range(c * (CH // P) + cf0 // P,
                                    (c + 1) * (CH // P)):
                        ptr = pts.tile([P, P], F32, tag="pt")
                        nc.tensor.transpose(ptr, vT_sb[:, tt * P:(tt + 1) * P], id_sb)
                        nc.any.tensor_copy(v_sb[:, tt, :], ptr)
                    if c == min(1, nch - 1) and abl_pfx:
                        # overlap the exact-prefix pass with chunks 2-3
                        emit_prefix()

            # ---------------- Phase A: attention
            with tc.tile_pool(name="oh", bufs=1) as ohpool, \
                 tc.tile_pool(name="wo", bufs=1) as wopool:
                ohT_sb = ohpool.tile([P, REP * s_len], BF16)
                woT_sb = wopool.tile([P, REP * D], BF16)
                for j in range(REP):
                    nc.sync.dma_start(out=woT_sb[:, j * D:(j + 1) * D],
                                      in_=woT[j * P:(j + 1) * P, :])

                with tc.tile_pool(name="ew", bufs=8) as epool, \
                     tc.tile_pool(name="mt", bufs=4) as tpool, \
                     tc.tile_pool(name="nrm", bufs=4) as npool, \
                     tc.tile_pool(name="fo", bufs=3) as fopool, \
                     tc.tile_pool(name="ps_s", bufs=2, space="PSUM") as pss, \
                     tc.tile_pool(name="ps_o", bufs=4, space="PSUM") as pso, \
                     tc.tile_pool(name="ps_b", bufs=1, space="PSUM") as psb, \
                     tc.tile_pool(name="ps_d", bufs=1, space="PSUM") as psd:
                    for c in range(nch):
                        npair = (c + 1) * (CH // P) // 2  # causal sk-tile pairs
                        od = [pso.tile([P, CH], F32, tag="od", name=f"od{c}_{h}")
                              for h in range(REP)]
                        dd = psd.tile([16, CH], F32, tag="dd")

                        def pair_geom(pr):
                            # union moving (sq) range of tiles (2pr, 2pr+1);
                            # diag pairs are masked over the union range,
                            # which also zeroes slot1's invalid columns.
                            f0 = max(0, 2 * pr * P - c * CH)
                            return f0, CH - f0, pr >= npair - 2

                        def emit_scores(pr):
                            f0, fn, diag = pair_geom(pr)
                            exact = (c == 0 and pr == 0 and abl_pfx)
                            es = []
                            for h in range(REP):
                                if exact:
                                    ep = epool.tile([P, 2, CH], BF16,
                                                    tag="eb", name=f"eb{h}")
                                else:
                                    ep = epool.tile([P, 2, CH], F8, tag="e",
                                                    name=f"e{c}_{pr}_{h}")
                                for s01 in (0, 1):
                                    t = 2 * pr + s01
                                    ss = pss.tile([P, CH], F32, tag="sc")
                                    nc.tensor.matmul(
                                        ss[:, 0:fn],
                                        qk_sb[:, 4 * s_len + t * P: 4 * s_len + (t + 1) * P],
                                        qk_sb[:, h * s_len + c * CH + f0: h * s_len + c * CH + f0 + fn],
                                        start=True, stop=True)
                                    if diag:
                                        off = (c * CH - t * P) + 384 + f0
                                        tmp = tpool.tile([P, CH], F32, tag="mt")
                                        nc.vector.scalar_tensor_tensor(
                                            tmp[:, 0:fn], ss[:, 0:fn], scale,
                                            mb_sb[:, off:off + fn],
                                            op0=mybir.AluOpType.mult,
                                            op1=mybir.AluOpType.add)
                                        # mb already carries the ESH shift
                                        nc.scalar.activation(
                                            ep[:, s01, 0:fn], tmp[:, 0:fn],
                                            mybir.ActivationFunctionType.Exp)
                                    else:
                                        nc.scalar.activation(
                                            ep[:, s01, 0:fn], ss[:, 0:fn],
                                            mybir.ActivationFunctionType.Exp,
                                            scale=scale, bias=esh_sb)
                                es.append(ep)
                            return es

                        def emit_odd(pr, es):
                            f0, fn, _ = pair_geom(pr)
                            if c == 0 and pr == 0 and abl_pfx:
                                # exact-prefix pair: bf16, non-DoubleRow
                                for h in range(REP):
                                    for s01 in (0, 1):
                                        nc.tensor.matmul(
                                            od[h][:, f0:f0 + fn],
                                            vb_sb[:, s01, :],
                                            es[h][:, s01, 0:fn],
                                            start=(s01 == 0), stop=False,
                                            skip_group_check=True)
                                for h in range(REP):
                                    for s01 in (0, 1):
                                        nc.tensor.matmul(
                                            dd[:, f0:f0 + fn],
                                            onehb_sb[:, h, :],
                                            es[h][:, s01, 0:fn],
                                            start=(h == 0 and s01 == 0),
                                            stop=False,
                                            skip_group_check=True)
                                return
                            if not abl_avdr:
                                for h in range(REP):
                                    for s01 in (0, 1):
                                        nc.tensor.matmul(
                                            od[h][:, f0:f0 + fn],
                                            v_sb[:, 2 * pr + s01, :],
                                            es[h][:, s01, 0:fn],
                                            start=(pr == 0 and s01 == 0),
                                            stop=(pr == npair - 1 and s01 == 1),
                                            skip_group_check=True)
                                for h in range(REP):
                                    for s01 in (0, 1):
                                        nc.tensor.matmul(
                                            dd[:, f0:f0 + fn],
                                            oneh8_sb[:, h, 0, :],
                                            es[h][:, s01, 0:fn],
                                            start=(pr == 0 and h == 0 and s01 == 0),
                                            stop=(pr == npair - 1 and
                                                  h == REP - 1 and s01 == 1),
                                            skip_group_check=True)
                                return
                            for h in range(REP):
                                nc.tensor.matmul(
                                    od[h][:, f0:f0 + fn],
                                    v_sb[:, 2 * pr:2 * pr + 2, :],
                                    es[h][:, :, 0:fn], start=(pr == 0),
                                    stop=(pr == npair - 1), perf_mode=DR,
                                    skip_group_check=(c == 0))
                            for h in range(REP):
                                nc.tensor.matmul(
                                    dd[:, f0:f0 + fn],
                                    oneh8_sb[:, h, :, :],
                                    es[h][:, :, 0:fn],
                                    start=(pr == 0 and h == 0),
                                    stop=(pr == npair - 1 and h == REP - 1),
                                    perf_mode=DR,
                                    skip_group_check=(c == 0))

                        # one-pair software pipeline: scores(p+1) emitted
                        # before od/dd(p) so each exp has a full PE block of
                        # slack before its e-tile is consumed
                        es_prev = emit_scores(0)
                        for pr in range(1, npair):
                            es = emit_scores(pr)
                            emit_odd(pr - 1, es_prev)
                            es_prev = es
                        emit_odd(npair - 1, es_prev)
                        for h in range(REP):
                            # free the od bank promptly (DVE: ACT is exp-bound)
                            nc.vector.tensor_copy(
                                ohT_sb[:, h * s_len + c * CH: h * s_len + (c + 1) * CH],
                                od[h])
                        r4_sb = npool.tile([REP, CH], F32R, tag="r")
                        with nc.allow_low_precision(
                                reason="f32r is 4-byte storage; rounding only"):
                            nc.vector.reciprocal(r4_sb, dd[0:REP, :])
                        for h in range(REP):
                            oh_c = ohT_sb[:, h * s_len + c * CH: h * s_len + (c + 1) * CH]
                            bb = psb.tile([P, CH], F32, tag="bb")
                            nc.tensor.matmul(bb, oneh4_sb[:, h * P:(h + 1) * P],
                                             r4_sb, start=True, stop=True)
                            nc.vector.tensor_mul(oh_c, oh_c, bb)

                        # ---- output projection for this chunk's s-tiles
                        # (overlaps the next chunk's attention; starts the
                        # out DMA early)
                        for st in range(c * (CH // P), (c + 1) * (CH // P)):
                            fo = fopool.tile([P, D], BF16, tag="fo",
                                             name=f"fo{st}")
                            for dc in range(D // CH):
                                pf = pss.tile([P, CH], F32, tag="sc",
                                              name=f"pf{st}_{dc}")
                                for j in range(REP):
                                    nc.tensor.matmul(
                                        pf,
                                        ohT_sb[:, j * s_len + st * P: j * s_len + (st + 1) * P],
                                        woT_sb[:, j * D + dc * CH: j * D + (dc + 1) * CH],
                                        start=(j == 0), stop=(j == REP - 1))
                                nc.any.tensor_copy(fo[:, dc * CH:(dc + 1) * CH], pf)
                                nc.sync.dma_start(
                                    out=out[st * P:(st + 1) * P, dc * CH:(dc + 1) * CH],
                                    in_=fo[:, dc * CH:(dc + 1) * CH])


    nc.compile()
    return nc


_PERM = np.concatenate([np.arange(0, DK, 2), np.arange(1, DK, 2)])  # evens, odds


def _f8(a):
    return np.clip(a, -240.0, 240.0).astype(ml_dtypes.float8_e4m3)


def _prep_device_inputs(x, freqs_cos, freqs_sin, wq_w, wq_b, wk_w, wk_b,
                        wv_w, wv_b, wo_w, s_len=S):
    """Host-side sharding + layout. Returns list of in_maps (len 8)."""
    f32 = np.float32
    nch = s_len // CH

    def wtile(rows, permute, sc):
        # rows: [128 outs, D ins] -> [p(in%128), kt, col(out)] fp8
        r = rows[_PERM] if permute else rows
        blk = np.ascontiguousarray(r.T * sc).reshape(ND, P, P)   # [kt, p, c]
        return blk.transpose(1, 0, 2).reshape(P, D)              # [p, kt*128+c]

    cs = np.ascontiguousarray(
        np.concatenate([freqs_cos[:s_len].T, freqs_sin[:s_len].T],
                       axis=0) / WSC, dtype=f32)
    scm = np.ascontiguousarray(
        np.concatenate([freqs_sin[:s_len].T, freqs_cos[:s_len].T],
                       axis=0) / WSC, dtype=f32)
    oneh8 = np.zeros((P, REP, 2, 16), dtype=f32)
    oneh4 = np.zeros((REP, REP * P), dtype=f32)
    onehb = np.zeros((P, REP, 16), dtype=f32)
    for h in range(REP):
        oneh8[:, h, :, h] = 1.0
        oneh4[h, h * P:(h + 1) * P] = 1.0
        onehb[:, h, h] = 1.0
    pp, xx = np.meshgrid(np.arange(P), np.arange(896), indexing="ij")
    mb = np.where(pp <= xx - 384, ESH, NEG).astype(f32)
    idm = np.eye(P, dtype=f32)

    in_maps = []
    for d in range(NDEV):
        b, g = d // HK, d % HK
        # x: [s, d] -> [p, c, kt, col] fp8
        xt = np.ascontiguousarray(x[b, :s_len].T).reshape(ND, P, nch, CH)
        xt = _f8(xt.transpose(1, 2, 0, 3)).reshape(P, nch * ND * CH)
        # exact-prefix x: [p, kt, col<TP] bf16
        xbp = np.ascontiguousarray(x[b, :TP].T).reshape(ND, P, TP)
        xbp = xbp.transpose(1, 0, 2).reshape(P, ND * TP)
        xbp = xbp.astype(ml_dtypes.bfloat16)
        wblk = np.empty((P, 6 * D), dtype=f32)
        for m in range(REP):
            h = g * REP + m
            wblk[:, m * D:(m + 1) * D] = wtile(wq_w[h * P:(h + 1) * P], True, WSC)
        wblk[:, 4 * D:5 * D] = wtile(wk_w[g * P:(g + 1) * P], True, WSC)
        wblk[:, 5 * D:6 * D] = wtile(wv_w[g * P:(g + 1) * P], False, 1.0)
        wot = np.concatenate(
            [np.ascontiguousarray(wo_w[:, (g * REP + j) * P:(g * REP + j + 1) * P].T)
             for j in range(REP)], axis=0).astype(ml_dtypes.bfloat16)
        bias = np.zeros((P, 6), dtype=f32)
        for m in range(REP):
            h = g * REP + m
            bias[:, m] = wq_b[h * P:(h + 1) * P][_PERM] * WSC
        bias[:, 4] = wk_b[g * P:(g + 1) * P][_PERM] * WSC
        bias[:, 5] = wv_b[g * P:(g + 1) * P]
        in_maps.append({
            "xT8": np.ascontiguousarray(xt), "W8": _f8(wblk), "woT": wot,
            "XB": xbp, "WB": wblk.astype(ml_dtypes.bfloat16),
            "ONEHB": onehb.reshape(P, REP * 16).astype(ml_dtypes.bfloat16),
            "CS": cs, "SC": scm, "MB": mb, "ID": idm,
            "BIAS": np.ascontiguousarray(bias),
            "BIAS2": np.ascontiguousarray(np.roll(bias, 64, axis=0)),
            "ONEH8": _f8(oneh8).reshape(P, REP * 2 * 16),
            "ONEH4": oneh4,
        })
    return in_maps


_CACHE = {}


def _get_nc(s_len=S):
    if s_len not in _CACHE:
        _CACHE[s_len] = _build(s_len)
    return _CACHE[s_len]


def kernel(x, freqs_cos, freqs_sin, wq_w, wq_b, wk_w, wk_b, wv_w, wv_b,
           wo_w, wo_b, _trace=False):
    x = np.asarray(x, dtype=np.float32)
    args = [np.asarray(a, dtype=np.float32) for a in
            (freqs_cos, freqs_sin, wq_w, wq_b, wk_w, wk_b, wv_w, wv_b, wo_w)]
    wo_b = np.asarray(wo_b, dtype=np.float32)
    nc = _get_nc(S)
    in_maps = _prep_device_inputs(x, *args)
    res = run_bass_kernel_spmd(nc, in_maps, core_ids=list(range(NDEV)),
                               trace=_trace)
    outf = np.zeros((B, S, D), dtype=np.float32)
    for d in range(NDEV):
        outf[d // HK] += res.results[d]["out"].astype(np.float32)
    outf += wo_b[None, None, :]
    kernel.last_result = res
    return outf


# revision 36
# speedup vs baseline: 591187.5975x; 591187.5975x over previous
"""Trainium2 Bass kernel for GQA attention (B=2, S=2048, D=2048, H=16, HK=4).

Sharding: 8 devices = batch(2) x kv-groups(4). Each device owns one batch
element and one GQA group (4 q-heads + 1 kv-head): wq/wk/wv column-parallel,
wo row-parallel (host sums the 4 partials per batch element).

Device kernel:
  - QKV projection in fp8e4 DoubleRow (K=256 per matmul): x and the QKV
    weights are quantized host-side; the q/k weight blocks are pre-scaled
    by 128 (so fp8 never sees denormals) with the 1/128 folded into the
    RoPE cos/sin tables and biases. v's block is unscaled (its error is
    attenuated by softmax averaging).
  - RoPE on DVE with an even/odd dk permutation folded into the weights
    host-side (partitions 0:64 = real, 64:128 = imag); outputs bf16.
  - scores computed transposed [sk, sq] in bf16 so softmax needs no
    transposes; exp on ACT reads PSUM directly and writes fp8 e-tiles in
    sk-tile PAIRS; no max-subtraction (scores are O(1) here).
  - A@V and the denominator one-hot matmuls run fp8 DoubleRow over the
    sk-tile pairs (2x the fp32r rate, measured).
  - denominators: per-head one-hot DR matmuls accumulate row-sums into a
    single PSUM bank; one 4-row reciprocal; per-head broadcast via a K=4
    one-hot matmul.
  - causal: upper-triangle tiles skipped; diagonal pairs get an additive
    -1e9 mask before exp (which also zeroes the pair's invalid columns).
  - wo row-parallel fp32r matmul on device; host adds wo_b and reduces.
"""

import math

import ml_dtypes
import numpy as np

import concourse.bacc as bacc
import concourse.tile as tile
from concourse import mybir
from concourse.bass_utils import run_bass_kernel_spmd

B, S, D = 2, 2048, 2048
H, HK, DK = 16, 4, 128
REP = H // HK  # 4 q-heads per kv head
NDEV = 8
P = 128
CH = 512            # s-chunk (matmul moving size)
ND = D // P         # 16 d-tiles
NKP = ND // 2       # 8 DoubleRow k-pairs
F32 = mybir.dt.float32
F32R = mybir.dt.float32r
BF16 = mybir.dt.bfloat16
F8 = mybir.dt.float8e4
DR = mybir.MatmulPerfMode.DoubleRow
NEG = -1.0e9
WSC = 128.0         # host-side scale on q/k weight blocks (fp8 range use)
ESH = -2.0          # exp shift: e' = exp(s - 2); cancels in normalization.
                    # With k mean-centered, scores stay in [-8.3, 7.0], so
                    # e' <= ~160 < fp8e4 max 240 and row-maxes stay normal.
TP = 256            # exact-prefix length: the first TP query rows average
                    # too few positions to attenuate fp8 noise, so their
                    # q/k/v come from a bf16 projection and their attention
                    # pair runs in bf16.


_ABL = {"pfx": True, "ctr": True, "avdr": True}  # ablation knobs (timing expts)


def _build(s_len=S, reps=1):
    """Build the per-device Bass program (SPMD: same program on all cores).

    reps>1 repeats the whole computation (timing only)."""
    abl_pfx, abl_ctr, abl_avdr = _ABL["pfx"], _ABL["ctr"], _ABL["avdr"]
    nch = s_len // CH          # s-chunks
    scale = 1.0 / math.sqrt(DK)

    nc = bacc.Bacc("TRN2", target_bir_lowering=False, debug=False,
                   enable_asserts=False, num_devices=1)
    xT8 = nc.dram_tensor("xT8", [P, nch * ND * CH], F8, kind="ExternalInput").ap()
    W8 = nc.dram_tensor("W8", [P, 6 * D], F8, kind="ExternalInput").ap()
    XB = nc.dram_tensor("XB", [P, ND * TP], BF16, kind="ExternalInput").ap()
    WB = nc.dram_tensor("WB", [P, 6 * D], BF16, kind="ExternalInput").ap()
    ONEHB = nc.dram_tensor("ONEHB", [P, REP * 16], BF16, kind="ExternalInput").ap()
    woT = nc.dram_tensor("woT", [REP * DK, D], BF16, kind="ExternalInput").ap()
    CSt = nc.dram_tensor("CS", [P, s_len], F32, kind="ExternalInput").ap()
    SCt = nc.dram_tensor("SC", [P, s_len], F32, kind="ExternalInput").ap()
    MBt = nc.dram_tensor("MB", [P, 896], F32, kind="ExternalInput").ap()
    IDt = nc.dram_tensor("ID", [P, P], F32, kind="ExternalInput").ap()
    BIAS = nc.dram_tensor("BIAS", [P, 6], F32, kind="ExternalInput").ap()
    BIAS2 = nc.dram_tensor("BIAS2", [P, 6], F32, kind="ExternalInput").ap()
    ONEH8 = nc.dram_tensor("ONEH8", [P, REP * 2 * 16], F8, kind="ExternalInput").ap()
    ONEH4 = nc.dram_tensor("ONEH4", [REP, REP * P], F32R, kind="ExternalInput").ap()
    out = nc.dram_tensor("out", [s_len, D], BF16, kind="ExternalOutput").ap()

    with tile.TileContext(nc) as tc:
      for _rep in range(reps):
        with tc.tile_pool(name="consts", bufs=1) as consts, \
             tc.tile_pool(name="qkv", bufs=1) as qkpool:
            cs_sb = consts.tile([P, s_len], F32)
            sc_sb = consts.tile([P, s_len], F32)
            mb_sb = consts.tile([P, 896], F32)
            id_sb = consts.tile([P, P], F32)
            bias_sb = consts.tile([P, 6], F32)
            bias2_sb = consts.tile([P, 6], F32)
            oneh8_sb = consts.tile([P, REP, 2, 16], F8)
            oneh4_sb = consts.tile([REP, REP * P], F32R)
            esh_sb = consts.tile([P, 1], F32)
            nc.vector.memset(esh_sb, ESH)
            onehb_sb = consts.tile([P, REP, 16], BF16)

            qk_sb = qkpool.tile([P, 5 * s_len], BF16)  # roped q 0..3, k at 4
            v_sb = qkpool.tile([P, ND, P], F8)         # [s-in-tile, sk-tile, dk]
            vb_sb = qkpool.tile([P, TP // P, P], BF16)  # bf16 copy of v tiles 0-1

            # ---------------- Phase P: QKV projection (fp8 DR) + RoPE + vT
            with tc.tile_pool(name="xh", bufs=2) as xpool, \
                 tc.tile_pool(name="wst", bufs=1) as wpool, \
                 tc.tile_pool(name="rope", bufs=3) as rpool, \
                 tc.tile_pool(name="vT", bufs=1) as vtpool, \
                 tc.tile_pool(name="pp", bufs=4, space="PSUM") as pps, \
                 tc.tile_pool(name="pt", bufs=2, space="PSUM") as pts:
                vT_sb = vtpool.tile([P, s_len], F32)
                w8_sb = wpool.tile([P, 6, NKP, 2, P], F8)
                wb_sb = wpool.tile([P, 6, ND, P], BF16)
                xb_sb = wpool.tile([P, ND, TP], BF16)

                def load_xq(c):
                    xq = xpool.tile([P, NKP, 2, CH], F8, tag="x", name=f"xq{c}")
                    nc.sync.dma_start(
                        out=xq, in_=xT8[:, c * ND * CH:(c + 1) * ND * CH])
                    return xq

                def load_tabs(c):
                    # chunked so chunk-0 RoPE can start before the rest lands
                    nc.scalar.dma_start(out=cs_sb[:, c * CH:(c + 1) * CH],
                                        in_=CSt[:, c * CH:(c + 1) * CH])
                    nc.scalar.dma_start(out=sc_sb[:, c * CH:(c + 1) * CH],
                                        in_=SCt[:, c * CH:(c + 1) * CH])

                def emit_rope(ps, m, col0, fn):
                    # RoPE: partitions 0:64 real (qr), 64:128 imag.
                    # U[0:64]=(qr+b0)cos  U[64:]=(qr+b0)sin
                    # V[0:64]=(qi+b1)sin  V[64:]=(qi+b1)cos
                    # cos/sin tables carry the 1/WSC de-scale.
                    cs_c = cs_sb[:, col0:col0 + fn]
                    sc_c = sc_sb[:, col0:col0 + fn]
                    add, mult = mybir.AluOpType.add, mybir.AluOpType.mult
                    u = rpool.tile([P, CH], F32, tag="p1")
                    v = rpool.tile([P, CH], F32, tag="p2")
                    nc.vector.scalar_tensor_tensor(
                        u[0:64, 0:fn], ps[0:64, 0:fn], bias_sb[0:64, m:m + 1],
                        cs_c[0:64], op0=add, op1=mult)
                    nc.vector.scalar_tensor_tensor(
                        u[64:128, 0:fn], ps[0:64, 0:fn],
                        bias2_sb[64:128, m:m + 1],
                        cs_c[64:128], op0=add, op1=mult)
                    nc.vector.scalar_tensor_tensor(
                        v[0:64, 0:fn], ps[64:128, 0:fn],
                        bias2_sb[0:64, m:m + 1],
                        sc_c[0:64], op0=add, op1=mult)
                    nc.vector.scalar_tensor_tensor(
                        v[64:128, 0:fn], ps[64:128, 0:fn],
                        bias_sb[64:128, m:m + 1],
                        sc_c[64:128], op0=add, op1=mult)
                    dst = qk_sb[:, m * s_len + col0: m * s_len + col0 + fn]
                    nc.vector.tensor_sub(dst[0:64], u[0:64, 0:fn], v[0:64, 0:fn])
                    nc.vector.tensor_add(dst[64:128], u[64:128, 0:fn],
                                         v[64:128, 0:fn])

                nkm = wpool.tile([P, 1], F32)

                def center_k(col0, fn):
                    # scores_kj -> (k_k - kbar).q_j: a per-column shift that
                    # cancels in softmax normalization but tames the exp
                    # range so e' fits fp8e4. kbar is chunk-0's k mean (any
                    # fixed vector gives an exact-cancelling shift).
                    ksl = qk_sb[:, 4 * s_len + col0: 4 * s_len + col0 + fn]
                    nc.vector.tensor_scalar(ksl, ksl, nkm, None,
                                            op0=mybir.AluOpType.add)

                def emit_prefix():
                    # exact prefix: bf16 QKV for positions < TP overwrites
                    # the fp8-derived q/k/v there (short attention rows
                    # can't average away fp8 noise).
                    for m in range(6):
                        ps = pps.tile([P, TP], F32, tag="pp", name=f"pfx{m}")
                        for dt in range(ND):
                            nc.tensor.matmul(
                                ps, wb_sb[:, m, dt, :], xb_sb[:, dt, :],
                                start=(dt == 0), stop=(dt == ND - 1))
                        if m < 5:
                            emit_rope(ps, m, 0, TP)
                            if m == 4 and abl_ctr:
                                center_k(0, TP)
                        else:
                            nc.scalar.add(out=vT_sb[:, 0:TP], in_=ps,
                                          add=bias_sb[:, m:m + 1])
                    for tt in range(TP // P):
                        ptr = pts.tile([P, P], F32, tag="pt")
                        nc.tensor.transpose(ptr, vT_sb[:, tt * P:(tt + 1) * P],
                                            id_sb)
                        nc.any.tensor_copy(v_sb[:, tt, :], ptr)
                        nc.any.tensor_copy(vb_sb[:, tt, :], ptr)

                # W8 is only 1.5MB; x chunks are 1MB each. Interleave the
                # first x chunk with W8 so matmuls can start ~3us in.
                xq = load_xq(0)
                nc.scalar.dma_start(out=w8_sb, in_=W8)
                nc.scalar.dma_start(out=bias_sb, in_=BIAS)
                nc.scalar.dma_start(out=bias2_sb, in_=BIAS2)
                for cc in range(nch):
                    load_tabs(cc)
                nc.scalar.dma_start(out=id_sb, in_=IDt)

                for c in range(nch):
                    if c > 0:
                        xq = load_xq(c)
                    if c == (1 if nch > 1 else 0):
                        nc.sync.dma_start(out=mb_sb, in_=MBt)
                        nc.sync.dma_start(out=oneh8_sb, in_=ONEH8)
                        nc.sync.dma_start(out=oneh4_sb, in_=ONEH4)
                        nc.sync.dma_start(out=onehb_sb, in_=ONEHB)
                        if abl_pfx:
                            # prefix operands: needed once chunk 1 finishes
                            nc.scalar.dma_start(out=wb_sb, in_=WB)
                            nc.scalar.dma_start(out=xb_sb, in_=XB)
                    # chunk 0's first TP columns come from the exact prefix;
                    # skip them in the fp8 pass.
                    cf0 = TP if (c == 0 and abl_pfx) else 0
                    cw = CH - cf0
                    for m in range(6):
                        ps = pps.tile([P, CH], F32, tag="pp")
                        for kp in range(NKP):
                            nc.tensor.matmul(
                                ps[:, 0:cw], w8_sb[:, m, kp, :, :],
                                xq[:, kp, :, cf0:CH],
                                start=(kp == 0), stop=(kp == NKP - 1),
                                perf_mode=DR)
                        if m < 5:
                            emit_rope(ps, m, c * CH + cf0, cw)
                            if m == 4 and abl_ctr:
                                if c == 0:
                                    ksl = qk_sb[:, 4 * s_len + cf0:
                                                4 * s_len + CH]
                                    nc.vector.tensor_reduce(
                                        nkm, ksl, axis=mybir.AxisListType.X,
                                        op=mybir.AluOpType.add, negate=True)
                                    nc.vector.tensor_scalar(
                                        nkm, nkm, 1.0 / cw, None,
                                        op0=mybir.AluOpType.mult)
                                center_k(c * CH + cf0, cw)
                        else:
                            nc.scalar.add(out=vT_sb[:, c * CH + cf0:(c + 1) * CH],
                                          in_=ps[:, 0:cw],
                                          add=bias_sb[:, m:m + 1])
                    for tt in range(c * (CH // P) + cf0 // P,
                                    (c + 1) * (CH // P)):
                        ptr = pts.tile([P, P], F32, tag="pt")
                        nc.tensor.transpose(ptr, vT_sb[:, tt * P:(tt + 1) * P], id_sb)
                        nc.any.tensor_copy(v_sb[:, tt, :], ptr)
                    if c == min(2, nch - 1) and abl_pfx:
                        # overlap the exact-prefix pass with chunk 3
                        emit_prefix()

            # ---------------- Phase A: attention
            with tc.tile_pool(name="oh", bufs=1) as ohpool, \
                 tc.tile_pool(name="wo", bufs=1) as wopool:
                ohT_sb = ohpool.tile([P, REP * s_len], BF16)
                woT_sb = wopool.tile([P, REP * D], BF16)
                for j in range(REP):
                    nc.sync.dma_start(out=woT_sb[:, j * D:(j + 1) * D],
                                      in_=woT[j * P:(j + 1) * P, :])

                with tc.tile_pool(name="ew", bufs=8) as epool, \
                     tc.tile_pool(name="mt", bufs=4) as tpool, \
                     tc.tile_pool(name="nrm", bufs=4) as npool, \
                     tc.tile_pool(name="fo", bufs=3) as fopool, \
                     tc.tile_pool(name="ps_s", bufs=2, space="PSUM") as pss, \
                     tc.tile_pool(name="ps_o", bufs=4, space="PSUM") as pso, \
                     tc.tile_pool(name="ps_b", bufs=1, space="PSUM") as psb, \
                     tc.tile_pool(name="ps_d", bufs=1, space="PSUM") as psd:
                    for c in range(nch):
                        npair = (c + 1) * (CH // P) // 2  # causal sk-tile pairs
                        od = [pso.tile([P, CH], F32, tag="od", name=f"od{c}_{h}")
                              for h in range(REP)]
                        dd = psd.tile([16, CH], F32, tag="dd")

                        def pair_geom(pr):
                            # union moving (sq) range of tiles (2pr, 2pr+1);
                            # diag pairs are masked over the union range,
                            # which also zeroes slot1's invalid columns.
                            f0 = max(0, 2 * pr * P - c * CH)
                            return f0, CH - f0, pr >= npair - 2

                        def emit_scores(pr):
                            f0, fn, diag = pair_geom(pr)
                            exact = (c == 0 and pr == 0 and abl_pfx)
                            es = []
                            for h in range(REP):
                                if exact:
                                    ep = epool.tile([P, 2, CH], BF16,
                                                    tag="eb", name=f"eb{h}")
                                else:
                                    ep = epool.tile([P, 2, CH], F8, tag="e",
                                                    name=f"e{c}_{pr}_{h}")
                                for s01 in (0, 1):
                                    t = 2 * pr + s01
                                    ss = pss.tile([P, CH], F32, tag="sc")
                                    nc.tensor.matmul(
                                        ss[:, 0:fn],
                                        qk_sb[:, 4 * s_len + t * P: 4 * s_len + (t + 1) * P],
                                        qk_sb[:, h * s_len + c * CH + f0: h * s_len + c * CH + f0 + fn],
                                        start=True, stop=True)
                                    if diag:
                                        off = (c * CH - t * P) + 384 + f0
                                        tmp = tpool.tile([P, CH], F32, tag="mt")
                                        nc.vector.scalar_tensor_tensor(
                                            tmp[:, 0:fn], ss[:, 0:fn], scale,
                                            mb_sb[:, off:off + fn],
                                            op0=mybir.AluOpType.mult,
                                            op1=mybir.AluOpType.add)
                                        # mb already carries the ESH shift
                                        nc.scalar.activation(
                                            ep[:, s01, 0:fn], tmp[:, 0:fn],
                                            mybir.ActivationFunctionType.Exp)
                                    else:
                                        nc.scalar.activation(
                                            ep[:, s01, 0:fn], ss[:, 0:fn],
                                            mybir.ActivationFunctionType.Exp,
                                            scale=scale, bias=esh_sb)
                                es.append(ep)
                            return es

                        def emit_odd(pr, es):
                            f0, fn, _ = pair_geom(pr)
                            if c == 0 and pr == 0 and abl_pfx:
                                # exact-prefix pair: bf16, non-DoubleRow
                                for h in range(REP):
                                    for s01 in (0, 1):
                                        nc.tensor.matmul(
                                            od[h][:, f0:f0 + fn],
                                            vb_sb[:, s01, :],
                                            es[h][:, s01, 0:fn],
                                            start=(s01 == 0), stop=False,
                                            skip_group_check=True)
                                for h in range(REP):
                                    for s01 in (0, 1):
                                        nc.tensor.matmul(
                                            dd[:, f0:f0 + fn],
                                            onehb_sb[:, h, :],
                                            es[h][:, s01, 0:fn],
                                            start=(h == 0 and s01 == 0),
                                            stop=False,
                                            skip_group_check=True)
                                return
                            if not abl_avdr:
                                for h in range(REP):
                                    for s01 in (0, 1):
                                        nc.tensor.matmul(
                                            od[h][:, f0:f0 + fn],
                                            v_sb[:, 2 * pr + s01, :],
                                            es[h][:, s01, 0:fn],
                                            start=(pr == 0 and s01 == 0),
                                            stop=(pr == npair - 1 and s01 == 1),
                                            skip_group_check=True)
                                for h in range(REP):
                                    for s01 in (0, 1):
                                        nc.tensor.matmul(
                                            dd[:, f0:f0 + fn],
                                            oneh8_sb[:, h, 0, :],
                                            es[h][:, s01, 0:fn],
                                            start=(pr == 0 and h == 0 and s01 == 0),
                                            stop=(pr == npair - 1 and
                                                  h == REP - 1 and s01 == 1),
                                            skip_group_check=True)
                                return
                            for h in range(REP):
                                nc.tensor.matmul(
                                    od[h][:, f0:f0 + fn],
                                    v_sb[:, 2 * pr:2 * pr + 2, :],
                                    es[h][:, :, 0:fn], start=(pr == 0),
                                    stop=(pr == npair - 1), perf_mode=DR,
                                    skip_group_check=(c == 0))
                            for h in range(REP):
                                nc.tensor.matmul(
                                    dd[:, f0:f0 + fn],
                                    oneh8_sb[:, h, :, :],
                                    es[h][:, :, 0:fn],
                                    start=(pr == 0 and h == 0),
                                    stop=(pr == npair - 1 and h == REP - 1),
                                    perf_mode=DR,
                                    skip_group_check=(c == 0))

                        # one-pair software pipeline: scores(p+1) emitted
                        # before od/dd(p) so each exp has a full PE block of
                        # slack before its e-tile is consumed
                        es_prev = emit_scores(0)
                        for pr in range(1, npair):
                            es = emit_scores(pr)
                            emit_odd(pr - 1, es_prev)
                            es_prev = es
                        emit_odd(npair - 1, es_prev)
                        for h in range(REP):
                            # free the od bank promptly (DVE: ACT is exp-bound)
                            nc.vector.tensor_copy(
                                ohT_sb[:, h * s_len + c * CH: h * s_len + (c + 1) * CH],
                                od[h])
                        r4_sb = npool.tile([REP, CH], F32R, tag="r")
                        with nc.allow_low_precision(
                                reason="f32r is 4-byte storage; rounding only"):
                            nc.vector.reciprocal(r4_sb, dd[0:REP, :])
                        for h in range(REP):
                            oh_c = ohT_sb[:, h * s_len + c * CH: h * s_len + (c + 1) * CH]
                            bb = psb.tile([P, CH], F32, tag="bb")
                            nc.tensor.matmul(bb, oneh4_sb[:, h * P:(h + 1) * P],
                                             r4_sb, start=True, stop=True)
                            nc.vector.tensor_mul(oh_c, oh_c, bb)

                        # ---- output projection for this chunk's s-tiles
                        # (overlaps the next chunk's attention; starts the
                        # out DMA early)
                        for st in range(c * (CH // P), (c + 1) * (CH // P)):
                            fo = fopool.tile([P, D], BF16, tag="fo",
                                             name=f"fo{st}")
                            for dc in range(D // CH):
                                pf = pss.tile([P, CH], F32, tag="sc",
                                              name=f"pf{st}_{dc}")
                                for j in range(REP):
                                    nc.tensor.matmul(
                                        pf,
                                        ohT_sb[:, j * s_len + st * P: j * s_len + (st + 1) * P],
                                        woT_sb[:, j * D + dc * CH: j * D + (dc + 1) * CH],
                                        start=(j == 0), stop=(j == REP - 1))
                                nc.any.tensor_copy(fo[:, dc * CH:(dc + 1) * CH], pf)
                                nc.sync.dma_start(
                                    out=out[st * P:(st + 1) * P, dc * CH:(dc + 1) * CH],
                                    in_=fo[:, dc * CH:(dc + 1) * CH])


    nc.compile()
    return nc


_PERM = np.concatenate([np.arange(0, DK, 2), np.arange(1, DK, 2)])  # evens, odds


def _f8(a):
    return np.clip(a, -240.0, 240.0).astype(ml_dtypes.float8_e4m3)


def _prep_device_inputs(x, freqs_cos, freqs_sin, wq_w, wq_b, wk_w, wk_b,
                        wv_w, wv_b, wo_w, s_len=S):
    """Host-side sharding + layout. Returns list of in_maps (len 8)."""
    f32 = np.float32
    nch = s_len // CH

    def wtile(rows, permute, sc):
        # rows: [128 outs, D ins] -> [p(in%128), kt, col(out)] fp8
        r = rows[_PERM] if permute else rows
        blk = np.ascontiguousarray(r.T * sc).reshape(ND, P, P)   # [kt, p, c]
        return blk.transpose(1, 0, 2).reshape(P, D)              # [p, kt*128+c]

    cs = np.ascontiguousarray(
        np.concatenate([freqs_cos[:s_len].T, freqs_sin[:s_len].T],
                       axis=0) / WSC, dtype=f32)
    scm = np.ascontiguousarray(
        np.concatenate([freqs_sin[:s_len].T, freqs_cos[:s_len].T],
                       axis=0) / WSC, dtype=f32)
    oneh8 = np.zeros((P, REP, 2, 16), dtype=f32)
    oneh4 = np.zeros((REP, REP * P), dtype=f32)
    onehb = np.zeros((P, REP, 16), dtype=f32)
    for h in range(REP):
        oneh8[:, h, :, h] = 1.0
        oneh4[h, h * P:(h + 1) * P] = 1.0
        onehb[:, h, h] = 1.0
    pp, xx = np.meshgrid(np.arange(P), np.arange(896), indexing="ij")
    mb = np.where(pp <= xx - 384, ESH, NEG).astype(f32)
    idm = np.eye(P, dtype=f32)

    in_maps = []
    for d in range(NDEV):
        b, g = d // HK, d % HK
        # x: [s, d] -> [p, c, kt, col] fp8
        xt = np.ascontiguousarray(x[b, :s_len].T).reshape(ND, P, nch, CH)
        xt = _f8(xt.transpose(1, 2, 0, 3)).reshape(P, nch * ND * CH)
        # exact-prefix x: [p, kt, col<TP] bf16
        xbp = np.ascontiguousarray(x[b, :TP].T).reshape(ND, P, TP)
        xbp = xbp.transpose(1, 0, 2).reshape(P, ND * TP)
        xbp = xbp.astype(ml_dtypes.bfloat16)
        wblk = np.empty((P, 6 * D), dtype=f32)
        for m in range(REP):
            h = g * REP + m
            wblk[:, m * D:(m + 1) * D] = wtile(wq_w[h * P:(h + 1) * P], True, WSC)
        wblk[:, 4 * D:5 * D] = wtile(wk_w[g * P:(g + 1) * P], True, WSC)
        wblk[:, 5 * D:6 * D] = wtile(wv_w[g * P:(g + 1) * P], False, 1.0)
        wot = np.concatenate(
            [np.ascontiguousarray(wo_w[:, (g * REP + j) * P:(g * REP + j + 1) * P].T)
             for j in range(REP)], axis=0).astype(ml_dtypes.bfloat16)
        bias = np.zeros((P, 6), dtype=f32)
        for m in range(REP):
            h = g * REP + m
            bias[:, m] = wq_b[h * P:(h + 1) * P][_PERM] * WSC
        bias[:, 4] = wk_b[g * P:(g + 1) * P][_PERM] * WSC
        bias[:, 5] = wv_b[g * P:(g + 1) * P]
        in_maps.append({
            "xT8": np.ascontiguousarray(xt), "W8": _f8(wblk), "woT": wot,
            "XB": xbp, "WB": wblk.astype(ml_dtypes.bfloat16),
            "ONEHB": onehb.reshape(P, REP * 16).astype(ml_dtypes.bfloat16),
            "CS": cs, "SC": scm, "MB": mb, "ID": idm,
            "BIAS": np.ascontiguousarray(bias),
            "BIAS2": np.ascontiguousarray(np.roll(bias, 64, axis=0)),
            "ONEH8": _f8(oneh8).reshape(P, REP * 2 * 16),
            "ONEH4": oneh4,
        })
    return in_maps


_CACHE = {}


def _get_nc(s_len=S):
    if s_len not in _CACHE:
        _CACHE[s_len] = _build(s_len)
    return _CACHE[s_len]


def kernel(x, freqs_cos, freqs_sin, wq_w, wq_b, wk_w, wk_b, wv_w, wv_b,
           wo_w, wo_b, _trace=False):
    x = np.asarray(x, dtype=np.float32)
    args = [np.asarray(a, dtype=np.float32) for a in
            (freqs_cos, freqs_sin, wq_w, wq_b, wk_w, wk_b, wv_w, wv_b, wo_w)]
    wo_b = np.asarray(wo_b, dtype=np.float32)
    nc = _get_nc(S)
    in_maps = _prep_device_inputs(x, *args)
    res = run_bass_kernel_spmd(nc, in_maps, core_ids=list(range(NDEV)),
                               trace=_trace)
    outf = np.zeros((B, S, D), dtype=np.float32)
    for d in range(NDEV):
        outf[d // HK] += res.results[d]["out"].astype(np.float32)
    outf += wo_b[None, None, :]
    kernel.last_result = res
    return outf


# revision 38
# speedup vs baseline: 1046916.2584x; 1.7709x over previous
"""Trainium2 Bass kernel for GQA attention (B=2, S=2048, D=2048, H=16, HK=4).

Sharding: 8 devices = batch(2) x kv-groups(4). Each device owns one batch
element and one GQA group (4 q-heads + 1 kv-head): wq/wk/wv column-parallel,
wo row-parallel (host sums the 4 partials per batch element).

Device kernel:
  - QKV projection in fp8e4 DoubleRow (K=256 per matmul): x and the QKV
    weights are quantized host-side; the q/k weight blocks are pre-scaled
    by 128 (so fp8 never sees denormals) with the 1/128 folded into the
    RoPE cos/sin tables and biases. v's block is unscaled (its error is
    attenuated by softmax averaging).
  - RoPE on DVE with an even/odd dk permutation folded into the weights
    host-side (partitions 0:64 = real, 64:128 = imag); outputs bf16.
  - scores computed transposed [sk, sq] in bf16 so softmax needs no
    transposes; exp on ACT reads PSUM directly and writes fp8 e-tiles in
    sk-tile PAIRS; no max-subtraction (scores are O(1) here).
  - A@V and the denominator one-hot matmuls run fp8 DoubleRow over the
    sk-tile pairs (2x the fp32r rate, measured).
  - denominators: per-head one-hot DR matmuls accumulate row-sums into a
    single PSUM bank; one 4-row reciprocal; per-head broadcast via a K=4
    one-hot matmul.
  - causal: upper-triangle tiles skipped; diagonal pairs get an additive
    -1e9 mask before exp (which also zeroes the pair's invalid columns).
  - wo row-parallel fp32r matmul on device; host adds wo_b and reduces.
"""

import math

import ml_dtypes
import numpy as np

import concourse.bacc as bacc
import concourse.tile as tile
from concourse import mybir
from concourse.bass_utils import run_bass_kernel_spmd

B, S, D = 2, 2048, 2048
H, HK, DK = 16, 4, 128
REP = H // HK  # 4 q-heads per kv head
NDEV = 8
P = 128
CH = 512            # s-chunk (matmul moving size)
ND = D // P         # 16 d-tiles
NKP = ND // 2       # 8 DoubleRow k-pairs
F32 = mybir.dt.float32
F32R = mybir.dt.float32r
BF16 = mybir.dt.bfloat16
F8 = mybir.dt.float8e4
DR = mybir.MatmulPerfMode.DoubleRow
NEG = -1.0e9
WSC = 128.0         # host-side scale on q/k weight blocks (fp8 range use)
ESH = -2.0          # exp shift: e' = exp(s - 2); cancels in normalization.
                    # With k mean-centered, scores stay in [-8.3, 7.0], so
                    # e' <= ~160 < fp8e4 max 240 and row-maxes stay normal.
TP = 256            # exact-prefix length: the first TP query rows average
                    # too few positions to attenuate fp8 noise, so their
                    # q/k/v come from a bf16 projection and their attention
                    # pair runs in bf16.


_ABL = {"pfx": True, "ctr": True, "avdr": True}  # ablation knobs (timing expts)


def _build(s_len=S, reps=1):
    """Build the per-device Bass program (SPMD: same program on all cores).

    reps>1 repeats the whole computation (timing only)."""
    abl_pfx, abl_ctr, abl_avdr = _ABL["pfx"], _ABL["ctr"], _ABL["avdr"]
    nch = s_len // CH          # s-chunks
    scale = 1.0 / math.sqrt(DK)

    nc = bacc.Bacc("TRN2", target_bir_lowering=False, debug=False,
                   enable_asserts=False, num_devices=1)
    xT8 = nc.dram_tensor("xT8", [P, nch * ND * CH], F8, kind="ExternalInput").ap()
    W8 = nc.dram_tensor("W8", [P, 6 * D], F8, kind="ExternalInput").ap()
    XB = nc.dram_tensor("XB", [P, ND * TP], BF16, kind="ExternalInput").ap()
    WB = nc.dram_tensor("WB", [P, 6 * D], BF16, kind="ExternalInput").ap()
    ONEHB = nc.dram_tensor("ONEHB", [P, REP * 16], BF16, kind="ExternalInput").ap()
    woT = nc.dram_tensor("woT", [REP * DK, D], BF16, kind="ExternalInput").ap()
    CSt = nc.dram_tensor("CS", [P, s_len], F32, kind="ExternalInput").ap()
    SCt = nc.dram_tensor("SC", [P, s_len], F32, kind="ExternalInput").ap()
    MBt = nc.dram_tensor("MB", [P, 896], F32, kind="ExternalInput").ap()
    IDt = nc.dram_tensor("ID", [P, P], F32, kind="ExternalInput").ap()
    BIAS = nc.dram_tensor("BIAS", [P, 6], F32, kind="ExternalInput").ap()
    BIAS2 = nc.dram_tensor("BIAS2", [P, 6], F32, kind="ExternalInput").ap()
    ONEH8 = nc.dram_tensor("ONEH8", [P, REP * 2 * 16], F8, kind="ExternalInput").ap()
    ONEH4 = nc.dram_tensor("ONEH4", [REP, REP * P], F32R, kind="ExternalInput").ap()
    out = nc.dram_tensor("out", [s_len, D], BF16, kind="ExternalOutput").ap()

    with tile.TileContext(nc) as tc:
      for _rep in range(reps):
        with tc.tile_pool(name="consts", bufs=1) as consts, \
             tc.tile_pool(name="qkv", bufs=1) as qkpool:
            cs_sb = consts.tile([P, s_len], F32)
            sc_sb = consts.tile([P, s_len], F32)
            mb_sb = consts.tile([P, 896], F32)
            id_sb = consts.tile([P, P], F32)
            bias_sb = consts.tile([P, 6], F32)
            bias2_sb = consts.tile([P, 6], F32)
            oneh8_sb = consts.tile([P, REP, 2, 16], F8)
            oneh4_sb = consts.tile([REP, REP * P], F32R)
            esh_sb = consts.tile([P, 1], F32)
            nc.vector.memset(esh_sb, ESH)
            onehb_sb = consts.tile([P, REP, 16], BF16)

            qk_sb = qkpool.tile([P, 5 * s_len], BF16)  # roped q 0..3, k at 4
            v_sb = qkpool.tile([P, ND, P], F8)         # [s-in-tile, sk-tile, dk]
            vb_sb = qkpool.tile([P, TP // P, P], BF16)  # bf16 copy of v tiles 0-1

            # ---------------- Phase P: QKV projection (fp8 DR) + RoPE + vT
            with tc.tile_pool(name="xh", bufs=2) as xpool, \
                 tc.tile_pool(name="wst", bufs=1) as wpool, \
                 tc.tile_pool(name="rope", bufs=3) as rpool, \
                 tc.tile_pool(name="vT", bufs=1) as vtpool, \
                 tc.tile_pool(name="pp", bufs=4, space="PSUM") as pps, \
                 tc.tile_pool(name="pt", bufs=2, space="PSUM") as pts:
                vT_sb = vtpool.tile([P, s_len], F32)
                w8_sb = wpool.tile([P, 6, NKP, 2, P], F8)
                wb_sb = wpool.tile([P, 6, ND, P], BF16)
                xb_sb = wpool.tile([P, ND, TP], BF16)

                def load_xq(c):
                    xq = xpool.tile([P, NKP, 2, CH], F8, tag="x", name=f"xq{c}")
                    nc.sync.dma_start(
                        out=xq, in_=xT8[:, c * ND * CH:(c + 1) * ND * CH])
                    return xq

                def load_tabs(c):
                    # chunked so chunk-0 RoPE can start before the rest lands
                    nc.scalar.dma_start(out=cs_sb[:, c * CH:(c + 1) * CH],
                                        in_=CSt[:, c * CH:(c + 1) * CH])
                    nc.scalar.dma_start(out=sc_sb[:, c * CH:(c + 1) * CH],
                                        in_=SCt[:, c * CH:(c + 1) * CH])

                def emit_rope(ps, m, col0, fn):
                    # RoPE: partitions 0:64 real (qr), 64:128 imag.
                    # U[0:64]=(qr+b0)cos  U[64:]=(qr+b0)sin
                    # V[0:64]=(qi+b1)sin  V[64:]=(qi+b1)cos
                    # cos/sin tables carry the 1/WSC de-scale.
                    cs_c = cs_sb[:, col0:col0 + fn]
                    sc_c = sc_sb[:, col0:col0 + fn]
                    add, mult = mybir.AluOpType.add, mybir.AluOpType.mult
                    u = rpool.tile([P, CH], F32, tag="p1")
                    v = rpool.tile([P, CH], F32, tag="p2")
                    nc.vector.scalar_tensor_tensor(
                        u[0:64, 0:fn], ps[0:64, 0:fn], bias_sb[0:64, m:m + 1],
                        cs_c[0:64], op0=add, op1=mult)
                    nc.vector.scalar_tensor_tensor(
                        u[64:128, 0:fn], ps[0:64, 0:fn],
                        bias2_sb[64:128, m:m + 1],
                        cs_c[64:128], op0=add, op1=mult)
                    nc.vector.scalar_tensor_tensor(
                        v[0:64, 0:fn], ps[64:128, 0:fn],
                        bias2_sb[0:64, m:m + 1],
                        sc_c[0:64], op0=add, op1=mult)
                    nc.vector.scalar_tensor_tensor(
                        v[64:128, 0:fn], ps[64:128, 0:fn],
                        bias_sb[64:128, m:m + 1],
                        sc_c[64:128], op0=add, op1=mult)
                    dst = qk_sb[:, m * s_len + col0: m * s_len + col0 + fn]
                    nc.vector.tensor_sub(dst[0:64], u[0:64, 0:fn], v[0:64, 0:fn])
                    nc.vector.tensor_add(dst[64:128], u[64:128, 0:fn],
                                         v[64:128, 0:fn])

                nkm = wpool.tile([P, 1], F32)

                def center_k(col0, fn):
                    # scores_kj -> (k_k - kbar).q_j: a per-column shift that
                    # cancels in softmax normalization but tames the exp
                    # range so e' fits fp8e4. kbar is chunk-0's k mean (any
                    # fixed vector gives an exact-cancelling shift).
                    ksl = qk_sb[:, 4 * s_len + col0: 4 * s_len + col0 + fn]
                    nc.vector.tensor_scalar(ksl, ksl, nkm, None,
                                            op0=mybir.AluOpType.add)

                def emit_prefix():
                    # exact prefix: bf16 QKV for positions < TP overwrites
                    # the fp8-derived q/k/v there (short attention rows
                    # can't average away fp8 noise).
                    for m in range(6):
                        ps = pps.tile([P, TP], F32, tag="pp", name=f"pfx{m}")
                        for dt in range(ND):
                            nc.tensor.matmul(
                                ps, wb_sb[:, m, dt, :], xb_sb[:, dt, :],
                                start=(dt == 0), stop=(dt == ND - 1))
                        if m < 5:
                            emit_rope(ps, m, 0, TP)
                            if m == 4 and abl_ctr:
                                center_k(0, TP)
                        else:
                            nc.scalar.add(out=vT_sb[:, 0:TP], in_=ps,
                                          add=bias_sb[:, m:m + 1])
                    for tt in range(TP // P):
                        ptr = pts.tile([P, P], F32, tag="pt")
                        nc.tensor.transpose(ptr, vT_sb[:, tt * P:(tt + 1) * P],
                                            id_sb)
                        nc.any.tensor_copy(v_sb[:, tt, :], ptr)
                        nc.any.tensor_copy(vb_sb[:, tt, :], ptr)

                # W8 is only 1.5MB; x chunks are 1MB each. Interleave the
                # first x chunk with W8 so matmuls can start ~3us in.
                xq = load_xq(0)
                nc.scalar.dma_start(out=w8_sb, in_=W8)
                nc.scalar.dma_start(out=bias_sb, in_=BIAS)
                nc.scalar.dma_start(out=bias2_sb, in_=BIAS2)
                for cc in range(nch):
                    load_tabs(cc)
                nc.scalar.dma_start(out=id_sb, in_=IDt)

                for c in range(nch):
                    if c > 0:
                        xq = load_xq(c)
                    if c == (1 if nch > 1 else 0):
                        nc.sync.dma_start(out=mb_sb, in_=MBt)
                        nc.sync.dma_start(out=oneh8_sb, in_=ONEH8)
                        nc.sync.dma_start(out=oneh4_sb, in_=ONEH4)
                        nc.sync.dma_start(out=onehb_sb, in_=ONEHB)
                        if abl_pfx:
                            # prefix operands: needed once chunk 1 finishes
                            nc.scalar.dma_start(out=wb_sb, in_=WB)
                            nc.scalar.dma_start(out=xb_sb, in_=XB)
                    # chunk 0's first TP columns come from the exact prefix;
                    # skip them in the fp8 pass.
                    cf0 = TP if (c == 0 and abl_pfx) else 0
                    cw = CH - cf0
                    for m in range(6):
                        ps = pps.tile([P, CH], F32, tag="pp")
                        for kp in range(NKP):
                            nc.tensor.matmul(
                                ps[:, 0:cw], w8_sb[:, m, kp, :, :],
                                xq[:, kp, :, cf0:CH],
                                start=(kp == 0), stop=(kp == NKP - 1),
                                perf_mode=DR)
                        if m < 5:
                            emit_rope(ps, m, c * CH + cf0, cw)
                            if m == 4 and abl_ctr:
                                if c == 0:
                                    ksl = qk_sb[:, 4 * s_len + cf0:
                                                4 * s_len + CH]
                                    nc.vector.tensor_reduce(
                                        nkm, ksl, axis=mybir.AxisListType.X,
                                        op=mybir.AluOpType.add, negate=True)
                                    nc.vector.tensor_scalar(
                                        nkm, nkm, 1.0 / cw, None,
                                        op0=mybir.AluOpType.mult)
                                center_k(c * CH + cf0, cw)
                        else:
                            nc.scalar.add(out=vT_sb[:, c * CH + cf0:(c + 1) * CH],
                                          in_=ps[:, 0:cw],
                                          add=bias_sb[:, m:m + 1])
                    for tt in range(c * (CH // P) + cf0 // P,
                                    (c + 1) * (CH // P)):
                        ptr = pts.tile([P, P], F32, tag="pt")
                        nc.tensor.transpose(ptr, vT_sb[:, tt * P:(tt + 1) * P], id_sb)
                        nc.any.tensor_copy(v_sb[:, tt, :], ptr)
                    if c == min(2, nch - 1) and abl_pfx:
                        # overlap the exact-prefix pass with chunk 3
                        emit_prefix()

            # ---------------- Phase A: attention
            with tc.tile_pool(name="oh", bufs=1) as ohpool, \
                 tc.tile_pool(name="wo", bufs=1) as wopool:
                ohT_sb = ohpool.tile([P, REP * s_len], BF16)
                woT_sb = wopool.tile([P, REP * D], BF16)
                for j in range(REP):
                    nc.sync.dma_start(out=woT_sb[:, j * D:(j + 1) * D],
                                      in_=woT[j * P:(j + 1) * P, :])

                with tc.tile_pool(name="ew", bufs=8) as epool, \
                     tc.tile_pool(name="mt", bufs=4) as tpool, \
                     tc.tile_pool(name="nrm", bufs=4) as npool, \
                     tc.tile_pool(name="fo", bufs=3) as fopool, \
                     tc.tile_pool(name="ps_s", bufs=2, space="PSUM") as pss, \
                     tc.tile_pool(name="ps_o", bufs=4, space="PSUM") as pso, \
                     tc.tile_pool(name="ps_b", bufs=1, space="PSUM") as psb, \
                     tc.tile_pool(name="ps_d", bufs=1, space="PSUM") as psd:

                    def emit_oproj(cc):
                        for st in range(cc * (CH // P), (cc + 1) * (CH // P)):
                            fo = fopool.tile([P, D], BF16, tag="fo",
                                             name=f"fo{st}")
                            for dc in range(D // CH):
                                pf = pss.tile([P, CH], F32, tag="sc",
                                              name=f"pf{st}_{dc}")
                                for j in range(REP):
                                    nc.tensor.matmul(
                                        pf,
                                        ohT_sb[:, j * s_len + st * P: j * s_len + (st + 1) * P],
                                        woT_sb[:, j * D + dc * CH: j * D + (dc + 1) * CH],
                                        start=(j == 0), stop=(j == REP - 1))
                                nc.any.tensor_copy(fo[:, dc * CH:(dc + 1) * CH], pf)
                                nc.sync.dma_start(
                                    out=out[st * P:(st + 1) * P, dc * CH:(dc + 1) * CH],
                                    in_=fo[:, dc * CH:(dc + 1) * CH])

                    for c in range(nch):
                        npair = (c + 1) * (CH // P) // 2  # causal sk-tile pairs
                        od = [pso.tile([P, CH], F32, tag="od", name=f"od{c}_{h}")
                              for h in range(REP)]
                        dd = psd.tile([16, CH], F32, tag="dd")

                        def pair_geom(pr):
                            # union moving (sq) range of tiles (2pr, 2pr+1);
                            # diag pairs are masked over the union range,
                            # which also zeroes slot1's invalid columns.
                            f0 = max(0, 2 * pr * P - c * CH)
                            return f0, CH - f0, pr >= npair - 2

                        def emit_scores(pr):
                            f0, fn, diag = pair_geom(pr)
                            exact = (c == 0 and pr == 0 and abl_pfx)
                            es = []
                            for h in range(REP):
                                if exact:
                                    ep = epool.tile([P, 2, CH], BF16,
                                                    tag="eb", name=f"eb{h}")
                                else:
                                    ep = epool.tile([P, 2, CH], F8, tag="e",
                                                    name=f"e{c}_{pr}_{h}")
                                for s01 in (0, 1):
                                    t = 2 * pr + s01
                                    ss = pss.tile([P, CH], F32, tag="sc")
                                    nc.tensor.matmul(
                                        ss[:, 0:fn],
                                        qk_sb[:, 4 * s_len + t * P: 4 * s_len + (t + 1) * P],
                                        qk_sb[:, h * s_len + c * CH + f0: h * s_len + c * CH + f0 + fn],
                                        start=True, stop=True)
                                    if diag:
                                        off = (c * CH - t * P) + 384 + f0
                                        tmp = tpool.tile([P, CH], F32, tag="mt")
                                        nc.vector.scalar_tensor_tensor(
                                            tmp[:, 0:fn], ss[:, 0:fn], scale,
                                            mb_sb[:, off:off + fn],
                                            op0=mybir.AluOpType.mult,
                                            op1=mybir.AluOpType.add)
                                        # mb already carries the ESH shift
                                        nc.scalar.activation(
                                            ep[:, s01, 0:fn], tmp[:, 0:fn],
                                            mybir.ActivationFunctionType.Exp)
                                    else:
                                        nc.scalar.activation(
                                            ep[:, s01, 0:fn], ss[:, 0:fn],
                                            mybir.ActivationFunctionType.Exp,
                                            scale=scale, bias=esh_sb)
                                es.append(ep)
                            return es

                        def emit_odd(pr, es):
                            f0, fn, _ = pair_geom(pr)
                            if c == 0 and pr == 0 and abl_pfx:
                                # exact-prefix pair: bf16, non-DoubleRow
                                for h in range(REP):
                                    for s01 in (0, 1):
                                        nc.tensor.matmul(
                                            od[h][:, f0:f0 + fn],
                                            vb_sb[:, s01, :],
                                            es[h][:, s01, 0:fn],
                                            start=(s01 == 0), stop=False,
                                            skip_group_check=True)
                                for h in range(REP):
                                    for s01 in (0, 1):
                                        nc.tensor.matmul(
                                            dd[:, f0:f0 + fn],
                                            onehb_sb[:, h, :],
                                            es[h][:, s01, 0:fn],
                                            start=(h == 0 and s01 == 0),
                                            stop=False,
                                            skip_group_check=True)
                                return
                            if not abl_avdr:
                                for h in range(REP):
                                    for s01 in (0, 1):
                                        nc.tensor.matmul(
                                            od[h][:, f0:f0 + fn],
                                            v_sb[:, 2 * pr + s01, :],
                                            es[h][:, s01, 0:fn],
                                            start=(pr == 0 and s01 == 0),
                                            stop=(pr == npair - 1 and s01 == 1),
                                            skip_group_check=True)
                                for h in range(REP):
                                    for s01 in (0, 1):
                                        nc.tensor.matmul(
                                            dd[:, f0:f0 + fn],
                                            oneh8_sb[:, h, 0, :],
                                            es[h][:, s01, 0:fn],
                                            start=(pr == 0 and h == 0 and s01 == 0),
                                            stop=(pr == npair - 1 and
                                                  h == REP - 1 and s01 == 1),
                                            skip_group_check=True)
                                return
                            for h in range(REP):
                                nc.tensor.matmul(
                                    od[h][:, f0:f0 + fn],
                                    v_sb[:, 2 * pr:2 * pr + 2, :],
                                    es[h][:, :, 0:fn], start=(pr == 0),
                                    stop=(pr == npair - 1), perf_mode=DR,
                                    skip_group_check=(c == 0))
                            for h in range(REP):
                                nc.tensor.matmul(
                                    dd[:, f0:f0 + fn],
                                    oneh8_sb[:, h, :, :],
                                    es[h][:, :, 0:fn],
                                    start=(pr == 0 and h == 0),
                                    stop=(pr == npair - 1 and h == REP - 1),
                                    perf_mode=DR,
                                    skip_group_check=(c == 0))

                        # one-pair software pipeline: scores(p+1) emitted
                        # before od/dd(p) so each exp has a full PE block of
                        # slack before its e-tile is consumed
                        es_prev = emit_scores(0)
                        for pr in range(1, npair):
                            es = emit_scores(pr)
                            emit_odd(pr - 1, es_prev)
                            es_prev = es
                        emit_odd(npair - 1, es_prev)
                        for h in range(REP):
                            # free the od bank promptly (DVE: ACT is exp-bound)
                            nc.vector.tensor_copy(
                                ohT_sb[:, h * s_len + c * CH: h * s_len + (c + 1) * CH],
                                od[h])
                        r4_sb = npool.tile([REP, CH], F32R, tag="r")
                        with nc.allow_low_precision(
                                reason="f32r is 4-byte storage; rounding only"):
                            nc.vector.reciprocal(r4_sb, dd[0:REP, :])
                        if c > 0:
                            # previous chunk's output projection, emitted
                            # inside the reciprocal window so the PE has
                            # independent work while DVE computes 1/d (and
                            # the out DMA starts early)
                            emit_oproj(c - 1)
                        for h in range(REP):
                            oh_c = ohT_sb[:, h * s_len + c * CH: h * s_len + (c + 1) * CH]
                            bb = psb.tile([P, CH], F32, tag="bb")
                            nc.tensor.matmul(bb, oneh4_sb[:, h * P:(h + 1) * P],
                                             r4_sb, start=True, stop=True)
                            nc.vector.tensor_mul(oh_c, oh_c, bb)
                    emit_oproj(nch - 1)


    nc.compile()
    return nc


_PERM = np.concatenate([np.arange(0, DK, 2), np.arange(1, DK, 2)])  # evens, odds


def _f8(a):
    return np.clip(a, -240.0, 240.0).astype(ml_dtypes.float8_e4m3)


def _prep_device_inputs(x, freqs_cos, freqs_sin, wq_w, wq_b, wk_w, wk_b,
                        wv_w, wv_b, wo_w, s_len=S):
    """Host-side sharding + layout. Returns list of in_maps (len 8)."""
    f32 = np.float32
    nch = s_len // CH

    def wtile(rows, permute, sc):
        # rows: [128 outs, D ins] -> [p(in%128), kt, col(out)] fp8
        r = rows[_PERM] if permute else rows
        blk = np.ascontiguousarray(r.T * sc).reshape(ND, P, P)   # [kt, p, c]
        return blk.transpose(1, 0, 2).reshape(P, D)              # [p, kt*128+c]

    cs = np.ascontiguousarray(
        np.concatenate([freqs_cos[:s_len].T, freqs_sin[:s_len].T],
                       axis=0) / WSC, dtype=f32)
    scm = np.ascontiguousarray(
        np.concatenate([freqs_sin[:s_len].T, freqs_cos[:s_len].T],
                       axis=0) / WSC, dtype=f32)
    oneh8 = np.zeros((P, REP, 2, 16), dtype=f32)
    oneh4 = np.zeros((REP, REP * P), dtype=f32)
    onehb = np.zeros((P, REP, 16), dtype=f32)
    for h in range(REP):
        oneh8[:, h, :, h] = 1.0
        oneh4[h, h * P:(h + 1) * P] = 1.0
        onehb[:, h, h] = 1.0
    pp, xx = np.meshgrid(np.arange(P), np.arange(896), indexing="ij")
    mb = np.where(pp <= xx - 384, ESH, NEG).astype(f32)
    idm = np.eye(P, dtype=f32)

    in_maps = []
    for d in range(NDEV):
        b, g = d // HK, d % HK
        # x: [s, d] -> [p, c, kt, col] fp8
        xt = np.ascontiguousarray(x[b, :s_len].T).reshape(ND, P, nch, CH)
        xt = _f8(xt.transpose(1, 2, 0, 3)).reshape(P, nch * ND * CH)
        # exact-prefix x: [p, kt, col<TP] bf16
        xbp = np.ascontiguousarray(x[b, :TP].T).reshape(ND, P, TP)
        xbp = xbp.transpose(1, 0, 2).reshape(P, ND * TP)
        xbp = xbp.astype(ml_dtypes.bfloat16)
        wblk = np.empty((P, 6 * D), dtype=f32)
        for m in range(REP):
            h = g * REP + m
            wblk[:, m * D:(m + 1) * D] = wtile(wq_w[h * P:(h + 1) * P], True, WSC)
        wblk[:, 4 * D:5 * D] = wtile(wk_w[g * P:(g + 1) * P], True, WSC)
        wblk[:, 5 * D:6 * D] = wtile(wv_w[g * P:(g + 1) * P], False, 1.0)
        wot = np.concatenate(
            [np.ascontiguousarray(wo_w[:, (g * REP + j) * P:(g * REP + j + 1) * P].T)
             for j in range(REP)], axis=0).astype(ml_dtypes.bfloat16)
        bias = np.zeros((P, 6), dtype=f32)
        for m in range(REP):
            h = g * REP + m
            bias[:, m] = wq_b[h * P:(h + 1) * P][_PERM] * WSC
        bias[:, 4] = wk_b[g * P:(g + 1) * P][_PERM] * WSC
        bias[:, 5] = wv_b[g * P:(g + 1) * P]
        in_maps.append({
            "xT8": np.ascontiguousarray(xt), "W8": _f8(wblk), "woT": wot,
            "XB": xbp, "WB": wblk.astype(ml_dtypes.bfloat16),
            "ONEHB": onehb.reshape(P, REP * 16).astype(ml_dtypes.bfloat16),
            "CS": cs, "SC": scm, "MB": mb, "ID": idm,
            "BIAS": np.ascontiguousarray(bias),
            "BIAS2": np.ascontiguousarray(np.roll(bias, 64, axis=0)),
            "ONEH8": _f8(oneh8).reshape(P, REP * 2 * 16),
            "ONEH4": oneh4,
        })
    return in_maps


_CACHE = {}


def _get_nc(s_len=S):
    if s_len not in _CACHE:
        _CACHE[s_len] = _build(s_len)
    return _CACHE[s_len]


def kernel(x, freqs_cos, freqs_sin, wq_w, wq_b, wk_w, wk_b, wv_w, wv_b,
           wo_w, wo_b, _trace=False):
    x = np.asarray(x, dtype=np.float32)
    args = [np.asarray(a, dtype=np.float32) for a in
            (freqs_cos, freqs_sin, wq_w, wq_b, wk_w, wk_b, wv_w, wv_b, wo_w)]
    wo_b = np.asarray(wo_b, dtype=np.float32)
    nc = _get_nc(S)
    in_maps = _prep_device_inputs(x, *args)
    res = run_bass_kernel_spmd(nc, in_maps, core_ids=list(range(NDEV)),
                               trace=_trace)
    outf = np.zeros((B, S, D), dtype=np.float32)
    for d in range(NDEV):
        outf[d // HK] += res.results[d]["out"].astype(np.float32)
    outf += wo_b[None, None, :]
    kernel.last_result = res
    return outf
